# revision 12
# baseline (speedup 1.0000x reference)
"""AttentionLSTM Trainium2 kernel (8 NeuronCores, data-parallel over batch).

Reference (25 steps): attention over 25 support vectors -> LSTM cell.
Sharding: batch 512 -> 64 rows/core; weights replicated; no collectives.

The recurrence chaotically amplifies rounding noise (~1e4x over 25 steps), so
ALL compute is f32 (bf16/fp16 anywhere -> 5-15% final error vs the 2e-2 gate).
f32 W_r|W_hh (33.6 MB) exceeds SBUF, so weights stream from HBM every step,
double-buffered, hidden under the f32 matmuls.

Conventions (all exact-in-f32 scalings):
 - sigmoid(x) = 0.5*tanh(0.5x)+0.5; i/f/o weight rows pre-scaled by 0.5
   host-side so one tanh serves every gate.
 - h is stored doubled (h2 = 2h, exact *2): h2 = (tanh(o/2)+1)*tanh(c);
   compensated by W_hh pre-scale 0.5 and softmax exp at scale 0.5.
 - c stored doubled (c2 = 2c), tanh applied at scale 0.5.
 - Attention is dual-rail: partitions 0:64 = even s, 64:128 = odd s
   (s=25 padded to 26; pad slot's score pinned at -1e30 once at init).
"""

import numpy as np

import concourse.bass as bass
import concourse.mybir as mybir
import concourse.tile as tile
from concourse import bacc
from concourse.bass_utils import run_bass_kernel_spmd
from concourse.masks import make_identity

F32 = mybir.dt.float32
F32R = mybir.dt.float32r
AL = mybir.AluOpType
AF = mybir.ActivationFunctionType

B, S, F, H = 512, 25, 1024, 1024
NCORES = 8
BL = B // NCORES          # 64 batch rows per core
SD = 13                   # dual-rail s slots (2*13 = 26 >= 25)
G4 = 4 * H                # 4096 gate dim
NCH = G4 // 512           # 8 n-chunks of 512
KT = 16                   # k-tiles: 0-7 W_hh (h), 8-15 W_r (r)
NSTEP = S

_BUILT = None


def _build():
    nc = bacc.Bacc("TRN2", target_bir_lowering=False, debug=False,
                   num_devices=NCORES)

    wt_ext = nc.dram_tensor("wt", [128, KT * G4], F32, kind="ExternalInput")
    wx_ext = nc.dram_tensor("wx", [128, 9 * G4], F32, kind="ExternalInput")
    sup_ext = nc.dram_tensor("sup", [128, SD * H], F32, kind="ExternalInput")
    xv_ext = nc.dram_tensor("xv", [128, 9 * BL], F32, kind="ExternalInput")
    out_ext = nc.dram_tensor("out", [BL, H], F32, kind="ExternalOutput")

    with tile.TileContext(nc) as tc:
        with (
            tc.tile_pool(name="const", bufs=1) as constp,
            tc.tile_pool(name="supp", bufs=1) as supp,
            tc.tile_pool(name="state", bufs=1) as st,
            tc.tile_pool(name="scratch", bufs=1) as scratch,
            tc.tile_pool(name="rails", bufs=2) as rails,
            tc.tile_pool(name="wstream", bufs=3) as ws,
            tc.tile_pool(name="psg", bufs=8, space="PSUM") as psg,
        ):
            ident = constp.tile([128, 128], F32)
            make_identity(nc, ident[:, :])

            sup = supp.tile([128, SD * H], F32)
            nc.sync.dma_start(sup[:, :], sup_ext[:, :])
            xv = constp.tile([128, 9 * BL], F32)
            nc.sync.dma_start(xv[:, :], xv_ext[:, :])

            # ---- xWb = x_aug @ W_x_aug (+bias row), k-outer streaming ----
            xwb = st.tile([BL, G4], F32)
            xps = [psg.tile([BL, 512], F32, tag="ps", name=f"xps{n}") for n in range(NCH)]
            for k in range(9):
                wk = ws.tile([128, G4], F32, tag="wk")
                nc.sync.dma_start(wk[:, :], wx_ext[:, k * G4:(k + 1) * G4])
                for n in range(NCH):
                    nc.tensor.matmul(
                        xps[n][:, :],
                        lhsT=xv[:, k * BL:(k + 1) * BL],
                        rhs=wk[:, n * 512:(n + 1) * 512],
                        start=(k == 0), stop=(k == 8),
                    )
            for n in range(NCH):
                nc.scalar.copy(xwb[:, n * 512:(n + 1) * 512], xps[n][:, :])

            # ---- state ----
            h2d = st.tile([128, H], F32)
            nc.vector.memset(h2d[:, :], 0.0)
            c2 = st.tile([BL, H], F32)
            nc.vector.memset(c2[:, :], 0.0)
            scores = st.tile([128, SD], F32)
            nc.vector.memset(scores[:, :], 0.0)
            nc.vector.memset(scores[64:128, SD - 1:SD], -1e30)

            hT = st.tile([128, 8 * BL], F32)
            rT = st.tile([128, 8 * BL], F32)
            vtmp = st.tile([BL, H], F32)
            gsum = st.tile([BL, G4], F32)
            tch = st.tile([BL, G4], F32)
            t1 = st.tile([BL, H], F32)
            t2 = st.tile([BL, H], F32)
            tc_t = st.tile([BL, H], F32)
            r = st.tile([BL, H], F32)
            exps = st.tile([128, SD], F32)
            wgt = st.tile([128, SD], F32)
            mxd = st.tile([128, 1], F32)
            mxu = st.tile([64, 1], F32)
            nb = st.tile([128, 1], F32)
            sed = st.tile([128, 1], F32)
            seu = st.tile([64, 1], F32)
            rcp = st.tile([128, 1], F32)

            for t in range(NSTEP):
                last = (t == NSTEP - 1)

                # -- hT via PE transposes (one psum slot, freed fast) --
                ph = psg.tile([128, 512], F32, tag="ps")
                for k in range(8):
                    nc.tensor.transpose(
                        ph[:, k * BL:(k + 1) * BL],
                        h2d[0:64, k * 128:(k + 1) * 128],
                        ident[0:64, 0:64],
                    )
                nc.vector.tensor_copy(hT[:, :], ph[:, :])

                # -- W_hh stream + matmuls (overlap attention below) --
                gps = [psg.tile([BL, 512], F32, tag="ps", name=f"gps{n}") for n in range(NCH)]
                for k in range(8):
                    wk = ws.tile([128, G4], F32, tag="wk")
                    nc.sync.dma_start(wk[:, :],
                                      wt_ext[:, k * G4:(k + 1) * G4])
                    for n in range(NCH):
                        nc.tensor.matmul(
                            gps[n][:, :],
                            lhsT=hT[:, k * BL:(k + 1) * BL],
                            rhs=wk[:, n * 512:(n + 1) * 512],
                            start=(k == 0), stop=False,
                        )

                # -- attention scores (dual-rail STT, f32) --
                for k in range(SD):
                    if k < SD - 1:
                        junk = scratch.tile([128, H], F32, tag="sjunk")
                        nc.vector.scalar_tensor_tensor(
                            out=junk[:, :], in0=sup[:, k * H:(k + 1) * H],
                            scalar=1.0, in1=h2d[:, :],
                            op0=AL.mult, op1=AL.mult,
                            accum_out=scores[:, k:k + 1],
                        )
                    else:
                        junk = scratch.tile([64, H], F32, tag="sjunk1")
                        nc.vector.scalar_tensor_tensor(
                            out=junk[:, :], in0=sup[0:64, k * H:(k + 1) * H],
                            scalar=1.0, in1=h2d[0:64, :],
                            op0=AL.mult, op1=AL.mult,
                            accum_out=scores[0:64, k:k + 1],
                        )

                # -- softmax (scores hold 2*true; exp at scale 0.5) --
                nc.vector.tensor_reduce(mxd[:, :], scores[:, :],
                                        axis=mybir.AxisListType.X, op=AL.max)
                nc.vector.tensor_copy(mxu[:, :], mxd[64:128, :])
                nc.vector.tensor_tensor(out=mxd[0:64, :], in0=mxd[0:64, :],
                                        in1=mxu[:, :], op=AL.max)
                nc.vector.tensor_scalar(out=nb[0:64, :], in0=mxd[0:64, :],
                                        scalar1=-0.5, scalar2=None,
                                        op0=AL.mult)
                nc.vector.tensor_copy(nb[64:128, :], nb[0:64, :])
                nc.scalar.activation(exps[:, :], scores[:, :], AF.Exp,
                                     bias=nb[:, :], scale=0.5,
                                     accum_out=sed[:, :])
                nc.vector.tensor_copy(seu[:, :], sed[64:128, :])
                nc.vector.tensor_tensor(out=sed[0:64, :], in0=sed[0:64, :],
                                        in1=seu[:, :], op=AL.add)
                nc.vector.reciprocal(rcp[0:64, :], sed[0:64, :])
                nc.vector.tensor_copy(rcp[64:128, :], rcp[0:64, :])
                nc.vector.tensor_scalar(out=wgt[:, :], in0=exps[:, :],
                                        scalar1=rcp[:, :], scalar2=None,
                                        op0=AL.mult)

                # -- r = sum_s w_s * support_s (dual rails, f32) --
                rd_prev = None
                for k in range(SD):
                    rd = rails.tile([128, H], F32, tag="rrail")
                    if k == 0:
                        nc.vector.tensor_scalar(
                            out=rd[:, :], in0=sup[:, 0:H],
                            scalar1=wgt[:, 0:1], scalar2=None, op0=AL.mult)
                    else:
                        nc.vector.scalar_tensor_tensor(
                            out=rd[:, :], in0=sup[:, k * H:(k + 1) * H],
                            scalar=wgt[:, k:k + 1], in1=rd_prev[:, :],
                            op0=AL.mult, op1=AL.add)
                    rd_prev = rd
                rfold = scratch.tile([64, H], F32, tag="rfold")
                nc.vector.tensor_copy(rfold[:, :], rd_prev[64:128, :])
                nc.vector.tensor_tensor(out=r[:, :], in0=rd_prev[0:64, :],
                                        in1=rfold[:, :], op=AL.add)

                # -- rT via DVE 32x32 block transpose + cross-partition moves
                nc.vector.transpose(vtmp[:, :], r[:, :])
                for i in range(2):
                    for j in range(4):
                        nc.vector.tensor_copy(
                            rT[32 * j:32 * (j + 1), :]
                            .rearrange("p (k f) -> p k f", f=64)
                            [:, :, 32 * i:32 * (i + 1)],
                            vtmp[32 * i:32 * (i + 1), :]
                            .rearrange("p (k f) -> p k f", f=128)
                            [:, :, 32 * j:32 * (j + 1)],
                        )

                # -- W_r stream + matmuls --
                for k in range(8):
                    wk = ws.tile([128, G4], F32, tag="wk")
                    nc.sync.dma_start(wk[:, :],
                                      wt_ext[:, (8 + k) * G4:(9 + k) * G4])
                    for n in range(NCH):
                        nc.tensor.matmul(
                            gps[n][:, :],
                            lhsT=rT[:, k * BL:(k + 1) * BL],
                            rhs=wk[:, n * 512:(n + 1) * 512],
                            start=False, stop=(k == 7),
                        )

                # -- epilogue: gsum = psum + xwb; tch = tanh(gsum) --
                for n in range(NCH):
                    nc.vector.scalar_tensor_tensor(
                        out=gsum[:, n * 512:(n + 1) * 512], in0=gps[n][:, :],
                        scalar=1.0, in1=xwb[:, n * 512:(n + 1) * 512],
                        op0=AL.mult, op1=AL.add)
                    nc.scalar.activation(tch[:, n * 512:(n + 1) * 512],
                                         gsum[:, n * 512:(n + 1) * 512],
                                         AF.Tanh)

                # -- LSTM pointwise (c2 = 2c) --
                nc.vector.scalar_tensor_tensor(
                    out=t1[:, :], in0=tch[:, 0:H], scalar=1.0,
                    in1=tch[:, 2 * H:3 * H], op0=AL.add, op1=AL.mult)
                nc.vector.scalar_tensor_tensor(
                    out=t2[:, :], in0=tch[:, H:2 * H], scalar=1.0,
                    in1=c2[:, :], op0=AL.add, op1=AL.mult)
                nc.vector.scalar_tensor_tensor(
                    out=c2[:, :], in0=t2[:, :], scalar=0.5, in1=t1[:, :],
                    op0=AL.mult, op1=AL.add)
                nc.scalar.activation(tc_t[:, :], c2[:, :], AF.Tanh, scale=0.5)
                if not last:
                    nc.vector.scalar_tensor_tensor(
                        out=h2d[0:64, :], in0=tch[:, 3 * H:4 * H], scalar=1.0,
                        in1=tc_t[:, :], op0=AL.add, op1=AL.mult)
                    nc.vector.tensor_copy(h2d[64:128, :], h2d[0:64, :])
                else:
                    nc.vector.scalar_tensor_tensor(
                        out=t1[:, :], in0=tch[:, 3 * H:4 * H], scalar=1.0,
                        in1=tc_t[:, :], op0=AL.add, op1=AL.mult)
                    nc.vector.tensor_scalar(out=t1[:, :], in0=t1[:, :],
                                            scalar1=0.5, scalar2=None,
                                            op0=AL.mult)
                    nc.sync.dma_start(out_ext[:, :], t1[:, :])

    nc.compile()
    return nc


def _host_prep(x, support_embeddings, W_ih, W_hh, b_ih, b_hh):
    f32 = np.float32
    x = np.asarray(x, f32)
    support = np.asarray(support_embeddings, f32)
    W_ih = np.asarray(W_ih, f32)
    W_hh = np.asarray(W_hh, f32)
    bias = np.asarray(b_ih, f32) + np.asarray(b_hh, f32)

    ps = np.full((G4, 1), 0.5, f32)
    ps[2 * H:3 * H] = 1.0  # g rows unscaled

    W_x = W_ih[:, :F] * ps
    W_r = W_ih[:, F:] * ps
    W_h2 = W_hh * ps * 0.5          # extra 0.5: h stored doubled
    bias_s = bias * ps[:, 0]

    W_catT = np.concatenate([W_h2, W_r], axis=1).T      # (2048, 4096)
    wt = np.ascontiguousarray(
        W_catT.reshape(KT, 128, G4).transpose(1, 0, 2).reshape(128, KT * G4))

    W_xa = np.concatenate([W_x.T, bias_s[None, :]], axis=0)
    W_xa = np.concatenate(
        [W_xa, np.zeros((9 * 128 - W_xa.shape[0], G4), f32)], axis=0)
    wx = np.ascontiguousarray(
        W_xa.reshape(9, 128, G4).transpose(1, 0, 2).reshape(128, 9 * G4))

    in_maps = []
    for cid in range(NCORES):
        sl = slice(cid * BL, (cid + 1) * BL)
        xs = x[sl]
        sup_c = support[sl]

        supd = np.zeros((128, SD, H), f32)
        supd[0:64] = sup_c[:, 0::2, :]
        supd[64:128, :12] = sup_c[:, 1::2, :]
        supd = supd.reshape(128, SD * H)

        xa = np.zeros((9 * 128, BL), f32)
        xa[0:F] = xs.T
        xa[F] = 1.0
        xvc = np.ascontiguousarray(
            xa.reshape(9, 128, BL).transpose(1, 0, 2).reshape(128, 9 * BL))

        in_maps.append({"wt": wt, "wx": wx, "sup": supd, "xv": xvc})
    return in_maps


def _get_built():
    global _BUILT
    if _BUILT is None:
        _BUILT = _build()
    return _BUILT


def _run(inputs, trace=False, tmpdir=None):
    in_maps = _host_prep(**inputs)
    nc = _get_built()
    res = run_bass_kernel_spmd(nc, in_maps, core_ids=list(range(NCORES)),
                               trace=trace, tmpdir=tmpdir)
    out = np.concatenate([res.results[c]["out"] for c in range(NCORES)],
                         axis=0).astype(np.float32)
    return out, res


def kernel(**inputs) -> np.ndarray:
    out, _ = _run(inputs)
    return out


# revision 14
# speedup vs baseline: 1.0798x; 1.0798x over previous
"""AttentionLSTM Trainium2 kernel (8 NeuronCores, data-parallel over batch).

Reference (25 steps): attention over 25 support vectors -> LSTM cell.
Sharding: batch 512 -> 64 rows/core; weights replicated; no collectives.

The recurrence chaotically amplifies rounding noise (~1e4x over 25 steps), so
ALL compute is f32 (bf16/fp16 anywhere -> 5-15% final error vs the 2e-2 gate).
f32 W_r|W_hh (33.6 MB) exceeds SBUF, so weights stream from HBM every step,
double-buffered, hidden under the f32 matmuls.

Conventions (all exact-in-f32 scalings):
 - sigmoid(x) = 0.5*tanh(0.5x)+0.5; i/f/o weight rows pre-scaled by 0.5
   host-side so one tanh serves every gate.
 - h is stored doubled (h2 = 2h, exact *2): h2 = (tanh(o/2)+1)*tanh(c);
   compensated by W_hh pre-scale 0.5 and softmax exp at scale 0.5.
 - c stored doubled (c2 = 2c), tanh applied at scale 0.5.
 - Attention is dual-rail: partitions 0:64 = even s, 64:128 = odd s
   (s=25 padded to 26; pad slot's score pinned at -1e30 once at init).
"""

import numpy as np

import concourse.bass as bass
import concourse.mybir as mybir
import concourse.tile as tile
from concourse import bacc
from concourse.bass_utils import run_bass_kernel_spmd
from concourse.masks import make_identity

F32 = mybir.dt.float32
BF16 = mybir.dt.bfloat16
F32R = mybir.dt.float32r
AL = mybir.AluOpType
AF = mybir.ActivationFunctionType

B, S, F, H = 512, 25, 1024, 1024
NCORES = 8
BL = B // NCORES          # 64 batch rows per core
SD = 13                   # dual-rail s slots (2*13 = 26 >= 25)
G4 = 4 * H                # 4096 gate dim
NCH = G4 // 512           # 8 n-chunks of 512
KT = 16                   # k-tiles: 0-7 W_hh (h), 8-15 W_r (r)
NSTEP = S

_BUILT = None


def _build():
    nc = bacc.Bacc("TRN2", target_bir_lowering=False, debug=False,
                   num_devices=NCORES)

    wt_ext = nc.dram_tensor("wt", [128, KT * 2 * G4], BF16, kind="ExternalInput")
    wx_ext = nc.dram_tensor("wx", [128, 9 * G4], F32, kind="ExternalInput")
    sup_ext = nc.dram_tensor("sup", [128, SD * H], F32, kind="ExternalInput")
    xv_ext = nc.dram_tensor("xv", [128, 9 * BL], F32, kind="ExternalInput")
    out_ext = nc.dram_tensor("out", [BL, H], F32, kind="ExternalOutput")

    with tile.TileContext(nc) as tc:
        with (
            tc.tile_pool(name="const", bufs=1) as constp,
            tc.tile_pool(name="supp", bufs=1) as supp,
            tc.tile_pool(name="state", bufs=1) as st,
            tc.tile_pool(name="scratch", bufs=1) as scratch,
            tc.tile_pool(name="rails", bufs=2) as rails,
            tc.tile_pool(name="wstream", bufs=3) as ws,
            tc.tile_pool(name="psg", bufs=8, space="PSUM") as psg,
        ):
            ident = constp.tile([128, 128], F32)
            make_identity(nc, ident[:, :])

            sup = supp.tile([128, SD * H], F32)
            nc.sync.dma_start(sup[:, :], sup_ext[:, :])
            xv = constp.tile([128, 9 * BL], F32)
            nc.sync.dma_start(xv[:, :], xv_ext[:, :])

            # ---- xWb = x_aug @ W_x_aug (+bias row), k-outer streaming ----
            xwb = st.tile([BL, G4], F32)
            xps = [psg.tile([BL, 512], F32, tag="ps", name=f"xps{n}") for n in range(NCH)]
            for k in range(9):
                wk = ws.tile([128, G4], F32, tag="wk")
                nc.sync.dma_start(wk[:, :], wx_ext[:, k * G4:(k + 1) * G4])
                for n in range(NCH):
                    nc.tensor.matmul(
                        xps[n][:, :],
                        lhsT=xv[:, k * BL:(k + 1) * BL],
                        rhs=wk[:, n * 512:(n + 1) * 512],
                        start=(k == 0), stop=(k == 8),
                    )
            for n in range(NCH):
                nc.scalar.copy(xwb[:, n * 512:(n + 1) * 512], xps[n][:, :])

            # ---- state ----
            h2d = st.tile([128, H], F32)
            nc.vector.memset(h2d[:, :], 0.0)
            c2 = st.tile([BL, H], F32)
            nc.vector.memset(c2[:, :], 0.0)
            scores = st.tile([128, SD], F32)
            nc.vector.memset(scores[:, :], 0.0)
            nc.vector.memset(scores[64:128, SD - 1:SD], -1e30)

            rTf = st.tile([128, 8 * BL], F32)
            hT1 = st.tile([128, 8 * BL], BF16)
            hT2 = st.tile([128, 8 * BL], BF16)
            rT1 = st.tile([128, 8 * BL], BF16)
            rT2 = st.tile([128, 8 * BL], BF16)
            vtmp = st.tile([BL, H], F32)
            gsum = st.tile([BL, G4], F32)
            tch = st.tile([BL, G4], F32)
            t1 = st.tile([BL, H], F32)
            t2 = st.tile([BL, H], F32)
            tc_t = st.tile([BL, H], F32)
            r = st.tile([BL, H], F32)
            exps = st.tile([128, SD], F32)
            wgt = st.tile([128, SD], F32)
            mxd = st.tile([128, 1], F32)
            mxu = st.tile([64, 1], F32)
            nb = st.tile([128, 1], F32)
            sed = st.tile([128, 1], F32)
            seu = st.tile([64, 1], F32)
            rcp = st.tile([128, 1], F32)

            for t in range(NSTEP):
                last = (t == NSTEP - 1)

                # -- hT via PE transposes (one psum slot, freed fast) --
                ph = psg.tile([128, 512], F32, tag="ps")
                for k in range(8):
                    nc.tensor.transpose(
                        ph[:, k * BL:(k + 1) * BL],
                        h2d[0:64, k * 128:(k + 1) * 128],
                        ident[0:64, 0:64],
                    )
                nc.vector.tensor_copy(hT1[:, :], ph[:, :])
                nc.vector.scalar_tensor_tensor(
                    out=hT2[:, :], in0=hT1[:, :], scalar=-1.0, in1=ph[:, :],
                    op0=AL.mult, op1=AL.add)

                # -- W_hh stream + matmuls (overlap attention below) --
                gps = [psg.tile([BL, 512], F32, tag="ps", name=f"gps{n}") for n in range(NCH)]
                for k in range(8):
                    wk = ws.tile([128, 2 * G4], BF16, tag="wk")
                    nc.sync.dma_start(wk[:, :],
                                      wt_ext[:, k * 2 * G4:(k + 1) * 2 * G4])
                    for a_t, woff, first in ((hT1, 0, True), (hT1, G4, False),
                                             (hT2, 0, False)):
                        for n in range(NCH):
                            nc.tensor.matmul(
                                gps[n][:, :],
                                lhsT=a_t[:, k * BL:(k + 1) * BL],
                                rhs=wk[:, woff + n * 512:woff + (n + 1) * 512],
                                start=(k == 0 and first), stop=False,
                            )

                # -- attention scores (dual-rail STT, f32) --
                for k in range(SD):
                    if k < SD - 1:
                        junk = scratch.tile([128, H], F32, tag="sjunk")
                        nc.vector.scalar_tensor_tensor(
                            out=junk[:, :], in0=sup[:, k * H:(k + 1) * H],
                            scalar=1.0, in1=h2d[:, :],
                            op0=AL.mult, op1=AL.mult,
                            accum_out=scores[:, k:k + 1],
                        )
                    else:
                        junk = scratch.tile([64, H], F32, tag="sjunk1")
                        nc.vector.scalar_tensor_tensor(
                            out=junk[:, :], in0=sup[0:64, k * H:(k + 1) * H],
                            scalar=1.0, in1=h2d[0:64, :],
                            op0=AL.mult, op1=AL.mult,
                            accum_out=scores[0:64, k:k + 1],
                        )

                # -- softmax (scores hold 2*true; exp at scale 0.5) --
                nc.vector.tensor_reduce(mxd[:, :], scores[:, :],
                                        axis=mybir.AxisListType.X, op=AL.max)
                nc.vector.tensor_copy(mxu[:, :], mxd[64:128, :])
                nc.vector.tensor_tensor(out=mxd[0:64, :], in0=mxd[0:64, :],
                                        in1=mxu[:, :], op=AL.max)
                nc.vector.tensor_scalar(out=nb[0:64, :], in0=mxd[0:64, :],
                                        scalar1=-0.5, scalar2=None,
                                        op0=AL.mult)
                nc.vector.tensor_copy(nb[64:128, :], nb[0:64, :])
                nc.scalar.activation(exps[:, :], scores[:, :], AF.Exp,
                                     bias=nb[:, :], scale=0.5,
                                     accum_out=sed[:, :])
                nc.vector.tensor_copy(seu[:, :], sed[64:128, :])
                nc.vector.tensor_tensor(out=sed[0:64, :], in0=sed[0:64, :],
                                        in1=seu[:, :], op=AL.add)
                nc.vector.reciprocal(rcp[0:64, :], sed[0:64, :])
                nc.vector.tensor_copy(rcp[64:128, :], rcp[0:64, :])
                nc.vector.tensor_scalar(out=wgt[:, :], in0=exps[:, :],
                                        scalar1=rcp[:, :], scalar2=None,
                                        op0=AL.mult)

                # -- r = sum_s w_s * support_s (dual rails, f32) --
                rd_prev = None
                for k in range(SD):
                    rd = rails.tile([128, H], F32, tag="rrail")
                    if k == 0:
                        nc.vector.tensor_scalar(
                            out=rd[:, :], in0=sup[:, 0:H],
                            scalar1=wgt[:, 0:1], scalar2=None, op0=AL.mult)
                    else:
                        nc.vector.scalar_tensor_tensor(
                            out=rd[:, :], in0=sup[:, k * H:(k + 1) * H],
                            scalar=wgt[:, k:k + 1], in1=rd_prev[:, :],
                            op0=AL.mult, op1=AL.add)
                    rd_prev = rd
                rfold = scratch.tile([64, H], F32, tag="rfold")
                nc.vector.tensor_copy(rfold[:, :], rd_prev[64:128, :])
                nc.vector.tensor_tensor(out=r[:, :], in0=rd_prev[0:64, :],
                                        in1=rfold[:, :], op=AL.add)

                # -- rT via DVE 32x32 block transpose + cross-partition moves
                nc.vector.transpose(vtmp[:, :], r[:, :])
                for i in range(2):
                    for j in range(4):
                        nc.vector.tensor_copy(
                            rTf[32 * j:32 * (j + 1), :]
                            .rearrange("p (k f) -> p k f", f=64)
                            [:, :, 32 * i:32 * (i + 1)],
                            vtmp[32 * i:32 * (i + 1), :]
                            .rearrange("p (k f) -> p k f", f=128)
                            [:, :, 32 * j:32 * (j + 1)],
                        )

                nc.vector.tensor_copy(rT1[:, :], rTf[:, :])
                nc.vector.scalar_tensor_tensor(
                    out=rT2[:, :], in0=rT1[:, :], scalar=-1.0, in1=rTf[:, :],
                    op0=AL.mult, op1=AL.add)

                # -- W_r stream + matmuls --
                for k in range(8):
                    wk = ws.tile([128, 2 * G4], BF16, tag="wk")
                    nc.sync.dma_start(wk[:, :],
                                      wt_ext[:, (8 + k) * 2 * G4:(9 + k) * 2 * G4])
                    for a_t, woff, lastp in ((rT1, 0, False), (rT1, G4, False),
                                             (rT2, 0, True)):
                        for n in range(NCH):
                            nc.tensor.matmul(
                                gps[n][:, :],
                                lhsT=a_t[:, k * BL:(k + 1) * BL],
                                rhs=wk[:, woff + n * 512:woff + (n + 1) * 512],
                                start=False, stop=(k == 7 and lastp),
                            )

                # -- epilogue: gsum = psum + xwb; tch = tanh(gsum) --
                for n in range(NCH):
                    nc.vector.scalar_tensor_tensor(
                        out=gsum[:, n * 512:(n + 1) * 512], in0=gps[n][:, :],
                        scalar=1.0, in1=xwb[:, n * 512:(n + 1) * 512],
                        op0=AL.mult, op1=AL.add)
                    nc.scalar.activation(tch[:, n * 512:(n + 1) * 512],
                                         gsum[:, n * 512:(n + 1) * 512],
                                         AF.Tanh)

                # -- LSTM pointwise (c2 = 2c) --
                nc.vector.scalar_tensor_tensor(
                    out=t1[:, :], in0=tch[:, 0:H], scalar=1.0,
                    in1=tch[:, 2 * H:3 * H], op0=AL.add, op1=AL.mult)
                nc.vector.scalar_tensor_tensor(
                    out=t2[:, :], in0=tch[:, H:2 * H], scalar=1.0,
                    in1=c2[:, :], op0=AL.add, op1=AL.mult)
                nc.vector.scalar_tensor_tensor(
                    out=c2[:, :], in0=t2[:, :], scalar=0.5, in1=t1[:, :],
                    op0=AL.mult, op1=AL.add)
                nc.scalar.activation(tc_t[:, :], c2[:, :], AF.Tanh, scale=0.5)
                if not last:
                    nc.vector.scalar_tensor_tensor(
                        out=h2d[0:64, :], in0=tch[:, 3 * H:4 * H], scalar=1.0,
                        in1=tc_t[:, :], op0=AL.add, op1=AL.mult)
                    nc.vector.tensor_copy(h2d[64:128, :], h2d[0:64, :])
                else:
                    nc.vector.scalar_tensor_tensor(
                        out=t1[:, :], in0=tch[:, 3 * H:4 * H], scalar=1.0,
                        in1=tc_t[:, :], op0=AL.add, op1=AL.mult)
                    nc.vector.tensor_scalar(out=t1[:, :], in0=t1[:, :],
                                            scalar1=0.5, scalar2=None,
                                            op0=AL.mult)
                    nc.sync.dma_start(out_ext[:, :], t1[:, :])

    nc.compile()
    return nc


def _host_prep(x, support_embeddings, W_ih, W_hh, b_ih, b_hh):
    f32 = np.float32
    x = np.asarray(x, f32)
    support = np.asarray(support_embeddings, f32)
    W_ih = np.asarray(W_ih, f32)
    W_hh = np.asarray(W_hh, f32)
    bias = np.asarray(b_ih, f32) + np.asarray(b_hh, f32)

    ps = np.full((G4, 1), 0.5, f32)
    ps[2 * H:3 * H] = 1.0  # g rows unscaled

    W_x = W_ih[:, :F] * ps
    W_r = W_ih[:, F:] * ps
    W_h2 = W_hh * ps * 0.5          # extra 0.5: h stored doubled
    bias_s = bias * ps[:, 0]

    import ml_dtypes
    bf = ml_dtypes.bfloat16
    W_catT = np.concatenate([W_h2, W_r], axis=1).T      # (2048, 4096)
    wkt = W_catT.reshape(KT, 128, G4).transpose(1, 0, 2)   # (128, KT, G4)
    w_hi = wkt.astype(bf)
    w_lo = (wkt - w_hi.astype(f32)).astype(bf)
    wt = np.ascontiguousarray(
        np.concatenate([w_hi, w_lo], axis=2)               # (128, KT, 2*G4)
        .reshape(128, KT * 2 * G4))

    W_xa = np.concatenate([W_x.T, bias_s[None, :]], axis=0)
    W_xa = np.concatenate(
        [W_xa, np.zeros((9 * 128 - W_xa.shape[0], G4), f32)], axis=0)
    wx = np.ascontiguousarray(
        W_xa.reshape(9, 128, G4).transpose(1, 0, 2).reshape(128, 9 * G4))

    in_maps = []
    for cid in range(NCORES):
        sl = slice(cid * BL, (cid + 1) * BL)
        xs = x[sl]
        sup_c = support[sl]

        supd = np.zeros((128, SD, H), f32)
        supd[0:64] = sup_c[:, 0::2, :]
        supd[64:128, :12] = sup_c[:, 1::2, :]
        supd = supd.reshape(128, SD * H)

        xa = np.zeros((9 * 128, BL), f32)
        xa[0:F] = xs.T
        xa[F] = 1.0
        xvc = np.ascontiguousarray(
            xa.reshape(9, 128, BL).transpose(1, 0, 2).reshape(128, 9 * BL))

        in_maps.append({"wt": wt, "wx": wx, "sup": supd, "xv": xvc})
    return in_maps


def _get_built():
    global _BUILT
    if _BUILT is None:
        _BUILT = _build()
    return _BUILT


def _run(inputs, trace=False, tmpdir=None):
    in_maps = _host_prep(**inputs)
    nc = _get_built()
    res = run_bass_kernel_spmd(nc, in_maps, core_ids=list(range(NCORES)),
                               trace=trace, tmpdir=tmpdir)
    out = np.concatenate([res.results[c]["out"] for c in range(NCORES)],
                         axis=0).astype(np.float32)
    return out, res


def kernel(**inputs) -> np.ndarray:
    out, _ = _run(inputs)
    return out
